# revision 14
# baseline (speedup 1.0000x reference)
"""Trainium2 Bass kernel for nn_DualDescriptorTS.

Math:  Nk[b,i] = sum_{j,g} x[b,j] * P[i,j,g] * cos(2*pi*k[b]/p[i,j,g]),
       p[i,j,g] = i*1024 + j*16 + g + 2,  x = emb[token_indices].

Key identity (k = arange(B), so k_b = b = 32*h + l, h in [0,128),
l in [0,32)): by angle addition, the P-weighted phi slab of each
(i, j) pair is a small-rank product

  D_{i,j}[l, h] = stat^T mov,
  stat[(c,g), l] = {P*cos(l*th_g), -P*sin(l*th_g)},  mov[(c,g), h] =
  {cos(32h*th_g), sin(32h*th_g)},  th_g = 2*pi/p.

mov is P-independent and numerically low rank (the 16 periods of a slab
are nearly equal): mov ~ A @ Q with per-slab rank K (median 2, tau=1%
Frobenius tail).  Q ships as the matmul moving operand; P folds into
the tiny stationary factor A^T @ stat per call.

Device (per core, rows i in [8c, 8c+8)): groups of 4 slabs (one PSUM
slot) merge into ONE matmul with a block-diagonal stationary
[sumK, 4*32l] and stacked moving [sumK, 128h] -> 128 matmuls/core
filling PSUM [4*32 l-bands, 16 slots x 128 h] per row i.  The DVE
multiplies PSUM by the token embeddings (fp16), gpsimd + DVE fold the
16 slots to 4, and the host adds the last 4+4 partial groups.
"""
import numpy as np
import ml_dtypes

import concourse.bacc as bacc
import concourse.tile as tile
from concourse import mybir
from concourse.bass_utils import run_bass_kernel_spmd

F32 = mybir.dt.float32
BF16 = mybir.dt.bfloat16
FP16 = mybir.dt.float16
TWO_PI = 2.0 * np.pi

M, O, B = 64, 16, 4096
NCORES = 8
NI = 8            # i rows per core
NH, NL = 128, 32  # b = 32*h + l
TAU = 0.01        # relative Frobenius tail kept when truncating mov

_bf16 = ml_dtypes.bfloat16
_fp16 = np.float16
_nc_cache = {}
_last_results = None


def _factors():
    """P-independent per-slab SVD factors and the SPMD contract depths.

    mov[s] = A[s] @ Vt[s], slab s = 64*i_global + j.  SKL[i_loc][slot]
    is the contract depth of the merged matmul for (i_loc, slot) — the
    max over cores of the slot's summed slab ranks, so one program
    serves all cores.
    """
    if "fac" in _nc_cache:
        return _nc_cache["fac"]
    h = np.arange(NH, dtype=np.float64)
    ig = np.arange(M, dtype=np.float64)[:, None, None]
    jg = np.arange(M, dtype=np.float64)[None, :, None]
    gg = np.arange(O, dtype=np.float64)[None, None, :]
    theta = TWO_PI / (1024.0 * ig + 16.0 * jg + gg + 2.0)
    a1 = theta[..., None] * (32.0 * h)
    mov = np.concatenate([np.cos(a1), np.sin(a1)], axis=2).reshape(M * M, 32, NH)
    U, S, Vt = np.linalg.svd(mov.astype(np.float64), full_matrices=False)
    fro = np.sqrt((S ** 2).sum(1))
    tail = np.sqrt(np.cumsum((S ** 2)[:, ::-1], axis=1))[:, ::-1] / fro[:, None]
    Ks = np.maximum(
        np.array([np.searchsorted(-tail[s], -TAU) for s in range(M * M)]), 1)
    SK = Ks.reshape(M, 16, 4).sum(axis=2)                   # [i_glob, slot]
    SKL = SK.reshape(NCORES, NI, 16).max(axis=0)            # [i_loc, slot]
    A = (U * S[:, None, :]).astype(np.float32)              # [4096, 32, 32]
    fac = (A, Vt.astype(np.float32), Ks, SKL)
    _nc_cache["fac"] = fac
    return fac


def _build():
    if "nc" in _nc_cache:
        return _nc_cache["nc"]
    _, _, _, SKL = _factors()
    nc = bacc.Bacc(target_bir_lowering=False, debug=False)
    wt_d = nc.declare_dram_parameter("wt", [128, 16384], BF16, isOutput=False)
    vt_d = nc.declare_dram_parameter("vt", [128, 16384], BF16, isOutput=False)
    xa_d = nc.declare_dram_parameter("xa", [128, 2048], FP16, isOutput=False)
    out_d = nc.declare_dram_parameter("out", [1024, 512], FP16, isOutput=True)

    with tile.TileContext(nc) as tc:
        with (
            tc.tile_pool(name="xap", bufs=1) as xpool,
            tc.tile_pool(name="wv", bufs=3) as wpool,
            tc.tile_pool(name="tmp", bufs=3) as tpool,
            tc.tile_pool(name="red", bufs=3) as rpool,
            tc.tile_pool(name="ps", bufs=4, space="PSUM") as psum,
        ):
            iorder = [1, 2, 3, 4, 5, 6, 7, 0]
            xa = xpool.tile([128, 2048], FP16)
            wt_t, vt_t = {}, {}
            for n, i in enumerate(iorder):
                v = wpool.tile([128, 2048], BF16, name=f"vt{i}", tag="vt")
                w = wpool.tile([128, 2048], BF16, name=f"wt{i}", tag="wt")
                Ri = int(max(SKL[i]))
                if Ri > 32:
                    # fat row (i_loc 0): per-quarter transfers
                    for q in range(4):
                        Rq = int(max(SKL[i][4 * q:4 * q + 4]))
                        nc.sync.dma_start(
                            v[0:Rq, 512 * q:512 * (q + 1)],
                            vt_d[0:Rq, 2048 * i + 512 * q:
                                 2048 * i + 512 * (q + 1)])
                        nc.sync.dma_start(
                            w[0:Rq, 512 * q:512 * (q + 1)],
                            wt_d[0:Rq, 2048 * i + 512 * q:
                                 2048 * i + 512 * (q + 1)])
                else:
                    nc.sync.dma_start(v[0:Ri, :],
                                      vt_d[0:Ri, 2048 * i:2048 * (i + 1)])
                    nc.sync.dma_start(w[0:Ri, :],
                                      wt_d[0:Ri, 2048 * i:2048 * (i + 1)])
                if n == 0:
                    nc.sync.dma_start(xa[:], xa_d[:])
                vt_t[i] = v
                wt_t[i] = w

            for n, i in enumerate(iorder):
                # One matmul per PSUM slot: block-diag stationary merges
                # the slot's 4 slabs (output bands 32*ccol from the
                # stationary column blocks), moving rows stacked to match.
                tx = tpool.tile([128, 2048], FP16, name=f"tx{i}", tag="tx")
                for half in range(2):
                    ps = psum.tile([128, 1024], F32, tag="ps",
                                   name=f"ps{i}_{half}")
                    for sh in range(8):
                        slot = 8 * half + sh
                        K = int(SKL[i][slot])
                        nc.tensor.matmul(
                            ps[0:128, 128 * sh:128 * sh + 128],
                            vt_t[i][0:K, 128 * slot:128 * slot + 128],
                            wt_t[i][0:K, 128 * slot:128 * slot + 128],
                            start=True, stop=True)
                    nc.vector.tensor_tensor(
                        tx[:, 1024 * half:1024 * (half + 1)], ps[:, :],
                        xa[:, 1024 * half:1024 * (half + 1)],
                        mybir.AluOpType.mult)
                t1 = rpool.tile([128, 1024], FP16, name=f"t1_{i}", tag="t1")
                nc.gpsimd.tensor_tensor(t1[:], tx[:, 0:1024],
                                        tx[:, 1024:2048],
                                        mybir.AluOpType.add)
                t2 = rpool.tile([128, 512], FP16, name=f"t2_{i}", tag="t2")
                nc.vector.tensor_tensor(t2[:], t1[:, 0:512], t1[:, 512:1024],
                                        mybir.AluOpType.add)
                nc.sync.dma_start(out_d[128 * i:128 * (i + 1), :], t2[:])
    nc.compile()
    _nc_cache["nc"] = nc
    return nc


def _pack_tables(P_):
    """Per-core bf16 tables.  Per (i_loc, slot): stationary block for
    slab m sits at rows [rowoff, rowoff+K) x cols 128*slot + 32*m of vt;
    its moving rows at the same row range x cols 128*slot of wt."""
    A, Vt, Ks, SKL = _factors()
    l = np.arange(NL, dtype=np.float64)
    ig = np.arange(M, dtype=np.float64)[:, None, None]
    jg = np.arange(M, dtype=np.float64)[None, :, None]
    gg = np.arange(O, dtype=np.float64)[None, None, :]
    theta = TWO_PI / (1024.0 * ig + 16.0 * jg + gg + 2.0)
    a2 = theta[..., None] * l
    Pd = P_.astype(np.float64)
    stat = np.concatenate([Pd[..., None] * np.cos(a2),
                           -Pd[..., None] * np.sin(a2)],
                          axis=2).reshape(M * M, 32, NL).astype(np.float32)
    statp = np.matmul(A.transpose(0, 2, 1), stat)            # [4096,32,32]
    wts, vts = [], []
    for c in range(NCORES):
        wt = np.zeros((128, NI * 2048), dtype=_bf16)
        vt = np.zeros((128, NI * 2048), dtype=_bf16)
        for il in range(NI):
            for slot in range(16):
                rowoff = 0
                for m in range(4):
                    j = 4 * slot + m
                    s = (8 * c + il) * M + j
                    K = int(Ks[s])
                    col = 2048 * il + 128 * slot
                    wt[rowoff:rowoff + K, col:col + NH] = \
                        Vt[s][0:K].astype(_bf16)
                    vt[rowoff:rowoff + K, col + NL * m:col + NL * (m + 1)] = \
                        statp[s][0:K].astype(_bf16)
                    rowoff += K
        wts.append(wt)
        vts.append(vt)
    return wts, vts


def _pack_x(x):
    # xa[32*ccol + l, 128*s + h] = x[32h+l, j], j = 4*s + ccol
    x4 = x.reshape(NH, NL, 16, 4)                 # [h, l, s, ccol]
    xa = np.ascontiguousarray(x4.transpose(3, 1, 2, 0)).reshape(128, 2048)
    return xa.astype(_fp16)


def _numpy_fallback(k, x, P_):
    out = np.zeros((B, M), dtype=np.float32)
    periods = (np.arange(M * M * O, dtype=np.float32) + 2.0).reshape(M, M, O)
    CH = 256
    for s0 in range(0, B, CH):
        kb = k[s0:s0 + CH].astype(np.float32)
        phi = np.cos(np.float32(TWO_PI) * kb[:, None, None, None]
                     / periods[None]).astype(np.float32)
        out[s0:s0 + CH] = np.einsum('bj,ijg,bijg->bi', x[s0:s0 + CH],
                                    P_.astype(np.float32), phi,
                                    optimize=True).astype(np.float32)
    return out


def kernel(k_tensor, token_indices, emb, P):
    global _last_results
    k = np.asarray(k_tensor, dtype=np.float32).reshape(B)
    tok = np.asarray(token_indices).astype(np.int64).reshape(B)
    emb_ = np.asarray(emb, dtype=np.float32)
    P_ = np.asarray(P, dtype=np.float32)
    x = emb_[tok]                                          # [B, 64]

    if not np.array_equal(k, np.arange(B, dtype=np.float32)):
        return _numpy_fallback(k, x, P_)

    wts, vts = _pack_tables(P_)
    xa = _pack_x(x)
    nc = _build()
    in_maps = [{"wt": wts[c], "vt": vts[c], "xa": xa} for c in range(NCORES)]
    res = run_bass_kernel_spmd(nc, in_maps, list(range(NCORES)))
    _last_results = res
    out = np.empty((B, M), dtype=np.float32)
    for c in range(NCORES):
        od = res.results[c]["out"].astype(np.float32)      # [1024, 512]
        # row = 128*i_loc + 32*ccol + l; col = 128*sg + h (4 s-groups)
        acc = od.reshape(NI, 4, NL, 4, NH).sum(axis=(1, 3))  # [i_loc, l, h]
        out[:, 8 * c:8 * c + 8] = acc.transpose(2, 1, 0).reshape(B, NI)
    return out
